# revision 15
# baseline (speedup 1.0000x reference)
"""BinConv3d (sign-binarized 3x3x3 conv, NCDHW) on 8 Trainium2 NeuronCores.

Full inputs in, full output out. Sharding: depth dim D=64 split 8 ways
(8 output planes per core) with a 1-plane halo on the input; conv weights
and bias replicated to every core.

Host prep: each core's input slab is rearranged to [plane, quarter, ci,
34, 130] bf16 — the H dim split into 4 quarter-row panels of 32 rows,
each padded with 1 halo row above/below and 1 zero col left/right, so
the device needs no data reshuffling at all. (bf16 halves the input DMA;
it is sign-preserving, and the matmul consumes bf16 anyway.)

Per-core kernel — fully RAW Bass program (no tile framework): every
cross-engine dependency is an explicit semaphore, so no instruction
carries bookkeeping updates it doesn't need. The tensor queue sustains
~30ns/matmul (vs ~35ns with per-instruction updates).

  - Input: two half-panel DMAs per plane on the Scalar HW-DGE queue
    (launched one plane ahead), ScalarE Sign bf16 -> bf16 into per-
    plane sign slabs (one slab per half-plane, no reuse). Head: top
    halves ride the Scalar queue, bottom halves the Sync queue, so the
    first matmul starts after ~3 top-half loads.
  - Conv = 27 accumulating matmuls (K=32 ci, M=64 co, N=512) per 4-row
    output tile; 16-way PE tiling (quarter q on PE row-group 32q, even/
    odd 4-row blocks on PE column halves) = full 128x128 array. Only
    the 8 last-tap matmuls of a generation carry semaphore updates.
  - PSUM: raw 2-bank tensors, quarters 0,1 in A, 2,3 in B, x2 parity
    for double buffering = all 8 banks.
  - Drain: ScalarE drains all of A ([128, 1024] + bias), VectorE all
    of B, into 3-deep staging rings; one contiguous 512KB flush DMA
    each on the Sync queue (blocked DRAM layout, host untangles).
    In-order engine FIFOs make most buffer-reuse waits unnecessary.
"""

import numpy as np
import ml_dtypes

import concourse.bass as bass
import concourse.mybir as mybir
from concourse import bacc
from concourse.bass_utils import run_bass_kernel_spmd

CI = 32
CO = 64
D_FULL = 64
N_CORES = 8
D_OUT = D_FULL // N_CORES  # output planes per core
D_IN = D_OUT + 2  # input planes per core (1-plane halo each side)

_cache = {}


def build_conv_program(n_in_planes=D_IN, n_out_planes=D_OUT, H=128, W=128,
                       debug=False):
    """Build the per-core Bass program (SPMD: same program on all cores)."""
    f32 = mybir.dt.float32
    bf16 = mybir.dt.bfloat16
    Hq = H // 4          # rows per quarter-panel
    Hqp, Wp = Hq + 2, W + 2
    n_pairs = Hq // 8    # even/odd block pairs per quarter
    Hh = Hq // 2
    n_gens = n_out_planes * n_pairs
    assert Hq % 8 == 0 and W == 128

    nc = bacc.Bacc("TRN2", target_bir_lowering=False, debug=debug)
    x_in = nc.declare_dram_parameter(
        "xs", [n_in_planes, 4, CI, Hqp, Wp], bf16, isOutput=False)
    w_in = nc.declare_dram_parameter("wst", [128, 27, 2 * CO], bf16,
                                     isOutput=False)
    b_in = nc.declare_dram_parameter("bias", [128, 1], f32, isOutput=False)
    # blocked output layout: [d, pi, g, (h co), (qq r w)]; host untangles.
    # row = 32*(2g+qq) + 8pi + 4h + r
    y_out = nc.declare_dram_parameter(
        "y", [n_out_planes, n_pairs, 2, 128, 2 * 4 * W], f32, isOutput=True)

    wtr = nc.alloc_sbuf_tensor("wtr", [128, 27, 2 * CO], bf16)
    bsr = nc.alloc_sbuf_tensor("bsr", [128, 1], f32)
    xr = [nc.alloc_sbuf_tensor(f"xr{s}", [128, Hh + 2, Wp], bf16)
          for s in range(6)]
    sgn = [[nc.alloc_sbuf_tensor(f"sgn_{p}_{part}", [128, Hh + 2, Wp], bf16)
            for part in range(2)] for p in range(n_in_planes)]
    pts = {(ab, par): nc.alloc_psum_tensor(f"pt{ab}{par}", [128, 2, 512], f32)
           for ab in "AB" for par in range(2)}
    stgA = [nc.alloc_sbuf_tensor(f"stgA{j}", [128, 2 * 512], f32)
            for j in range(3)]
    stgB = [nc.alloc_sbuf_tensor(f"stgB{j}", [128, 2 * 512], f32)
            for j in range(3)]

    sW = nc.alloc_semaphore("sW")      # weights DMA done (+16)
    sBS = nc.alloc_semaphore("sBS")    # bias DMA done (+16)
    sXs = nc.alloc_semaphore("sXs")    # input DMAs, scalar queue (+16 ea)
    sXy = nc.alloc_semaphore("sXy")    # input DMAs, sync queue (+16 ea)
    sT = nc.alloc_semaphore("sT")      # top-half signs done (count)
    sB = nc.alloc_semaphore("sB")      # bottom-half signs done (count)
    sMM = nc.alloc_semaphore("sMM")    # PE pipeline drained post-gen (1/gen)
    sDA = nc.alloc_semaphore("sDA")    # scalar drains done (1/gen)
    sDB = nc.alloc_semaphore("sDB")    # vector drains done (1/gen)
    sFA = nc.alloc_semaphore("sFA")    # A flushes done (+16 ea)
    sFB = nc.alloc_semaphore("sFB")    # B flushes done (+16 ea)

    # ---- const loads (sync queue) ----
    nc.sync.dma_start(out=wtr[:], in_=w_in[:]).then_inc(
        sW, 16, skip_validation=True)
    nc.sync.dma_start(out=bsr[:], in_=b_in[:]).then_inc(
        sBS, 16, skip_validation=True)

    # ---- input loads: launch bookkeeping ----
    nxt = {"s": 0, "y": 0, "slot": 0}

    def launch_half(p, part, eng, sem, cnt_key):
        lo = 0 if part == 0 else Hh
        slot = nxt["slot"] % 6
        nxt["slot"] += 1
        nxt[cnt_key] += 1
        eng.dma_start(
            out=xr[slot][:],
            in_=x_in[p, :, :, lo:lo + Hh + 2].rearrange(
                "q c h w -> (q c) h w"),
        ).then_inc(sem, 16, skip_validation=True)
        return slot, nxt[cnt_key]

    pend = {}

    def sign_half(p, part):
        slot, cnt, sem = pend[(p, part)]
        nc.scalar.wait_ge(sem, 16 * cnt)
        nc.scalar.sign(sgn[p][part][:], xr[slot][:]).then_inc(
            sT if part == 0 else sB, 1, skip_validation=True)

    # head: tops on the scalar DGE queue, bottoms on sync (parallel)
    for p in range(3):
        pend[(p, 0)] = (*launch_half(p, 0, nc.scalar, sXs, "s"), sXs)
    for p in range(3):
        pend[(p, 1)] = (*launch_half(p, 1, nc.sync, sXy, "y"), sXy)
    for p in range(3):
        sign_half(p, 0)
    for p in range(3):
        sign_half(p, 1)

    for d in range(n_out_planes):
        prefetch = d + 3 < n_in_planes
        if prefetch:
            pend[(d + 3, 0)] = (*launch_half(d + 3, 0, nc.scalar, sXs, "s"),
                                sXs)
            pend[(d + 3, 1)] = (*launch_half(d + 3, 1, nc.scalar, sXs, "s"),
                                sXs)
        for pi in range(n_pairs):
            gi = d * n_pairs + pi
            par = gi % 2
            ptA, ptB = pts[("A", par)], pts[("B", par)]

            # -- tensor queue: gen-start waits + 216 matmuls --
            if pi == 0:
                nc.tensor.wait_ge(sT, d + 3)
                if d == 0:
                    nc.tensor.wait_ge(sW, 16)
            if pi == 2:
                nc.tensor.wait_ge(sB, d + 3)
            if gi >= 2:
                nc.tensor.wait_ge(sDA, gi - 1)
                nc.tensor.wait_ge(sDB, gi - 1)
            for tap in range(27):
                kd, r = divmod(tap, 9)
                kh, kw = divmod(r, 3)
                for q in range(4):
                    pt = ptA if q < 2 else ptB
                    for half in range(2):
                        blk = 2 * pi + half
                        top = blk < Hq // 8
                        sg = sgn[d + kd][0 if top else 1]
                        row = 4 * blk + kh - (0 if top else Hh)
                        nc.tensor.matmul(
                            pt[64 * half:64 * half + 64, q % 2, :],
                            lhsT=wtr[32 * q:32 * q + 32, tap, 0:CO],
                            rhs=sg[32 * q:32 * q + 32,
                                   row:row + 4, kw:kw + W],
                            start=(tap == 0),
                            stop=(tap == 26),
                            tile_position=(32 * q, 64 * half),
                            skip_group_check=True,
                        )
            # flush the PE pipeline so PSUM is committed before drains
            nc.tensor.drain().then_inc(sMM, 1, skip_validation=True)

            # -- vector queue: drain B half (+bias) --
            nc.vector.wait_ge(sMM, gi + 1)
            if gi == 0:
                nc.vector.wait_ge(sBS, 16)
            if gi >= 3:
                nc.vector.wait_ge(sFB, 16 * (gi - 2))
            nc.vector.tensor_scalar_add(
                out=stgB[gi % 3][:],
                in0=ptB[:].rearrange("p q n -> p (q n)"),
                scalar1=bsr[:],
            ).then_inc(sDB, 1, skip_validation=True)

            # -- scalar queue: drain A half (+bias) --
            nc.scalar.wait_ge(sMM, gi + 1)
            if gi == 0:
                nc.scalar.wait_ge(sBS, 16)
            if gi >= 3:
                nc.scalar.wait_ge(sFA, 16 * (gi - 2))
            nc.scalar.activation(
                stgA[gi % 3][:],
                ptA[:].rearrange("p q n -> p (q n)"),
                mybir.ActivationFunctionType.Identity,
                bias=bsr[:], scale=1.0,
            ).then_inc(sDA, 1, skip_validation=True)

            # -- sync queue: blocked flushes (contiguous 512KB each) --
            nc.sync.wait_ge(sDA, gi + 1)
            nc.sync.dma_start(out=y_out[d, pi, 0],
                              in_=stgA[gi % 3][:]).then_inc(
                sFA, 16, skip_validation=True)
            nc.sync.wait_ge(sDB, gi + 1)
            nc.sync.dma_start(out=y_out[d, pi, 1],
                              in_=stgB[gi % 3][:]).then_inc(
                sFB, 16, skip_validation=True)

            # interleave next plane's signs between drains so the scalar
            # FIFO never blocks a drain on a pending DMA
            if prefetch and pi < 2:
                sign_half(d + 3, pi)

    # program end: all flushes landed
    nc.sync.wait_ge(sFA, 16 * n_gens)
    nc.sync.wait_ge(sFB, 16 * n_gens)
    nc.all_engine_barrier()

    nc.compile()
    return nc


def _get_program():
    if "nc" not in _cache:
        _cache["nc"] = build_conv_program()
    return _cache["nc"]


def prep_weights(W, b):
    W = np.asarray(W, dtype=np.float32)
    b = np.asarray(b, dtype=np.float32)
    # wst[q*32+ci, kd*9+kh*3+kw, half*64+co] = W[co, ci, kd, kh, kw],
    # replicated over the 4 row groups and the 2 col halves
    wq = W.transpose(1, 2, 3, 4, 0).reshape(CI, 27, CO)
    wq2 = np.concatenate([wq, wq], axis=2)  # duplicate col halves
    wst = np.ascontiguousarray(
        np.broadcast_to(wq2[None], (4, CI, 27, 2 * CO)).reshape(128, 27, 2 * CO)
    ).astype(ml_dtypes.bfloat16)
    bias = np.ascontiguousarray(
        np.concatenate([b, b]).reshape(128, 1).astype(np.float32))
    return wst, bias


def prep_x_slab(xpad, p_lo, n_planes, H=128, W=128):
    """xpad: [CI, D+2, H+2, W+2] zero-padded bf16 input. Returns
    [n_planes, 4, CI, H//4+2, W+2] bf16 slab for planes p_lo..p_lo+n_planes."""
    Hq = H // 4
    out = np.empty((n_planes, 4, CI, Hq + 2, W + 2), dtype=ml_dtypes.bfloat16)
    for q in range(4):
        # padded rows 32q .. 32q+34 cover global rows 32q-1 .. 32q+33
        out[:, q] = xpad[:, p_lo:p_lo + n_planes,
                         Hq * q:Hq * q + Hq + 2, :].transpose(1, 0, 2, 3)
    return out


def _prep_inputs(x, W, b):
    x = np.asarray(x, dtype=np.float32)
    wst, bias = prep_weights(W, b)
    xb = x[0].astype(ml_dtypes.bfloat16)
    xpad = np.pad(xb, ((0, 0), (1, 1), (1, 1), (1, 1)))
    in_maps = []
    for k in range(N_CORES):
        xs = prep_x_slab(xpad, D_OUT * k, D_IN)
        in_maps.append({"xs": xs, "wst": wst, "bias": bias})
    return in_maps


def _unblock_y(y8, H=128, W=128):
    """[d, pi, g, (h co), (qq r w)] -> [co, d, H, W];
    row = 32*(2g+qq) + 8pi + 4h + r."""
    nd = y8.shape[0]
    y = y8.reshape(nd, 4, 2, 2, CO, 2, 4, W)  # d,pi,g,h,co,qq,r,w
    y = np.transpose(y, (4, 0, 2, 5, 1, 3, 6, 7))  # co,d,g,qq,pi,h,r,w
    return np.ascontiguousarray(y).reshape(CO, nd, H, W)


def run(x, W, b, trace=False):
    """Run the kernel; returns (output, BassKernelResults)."""
    nc = _get_program()
    in_maps = _prep_inputs(x, W, b)
    res = run_bass_kernel_spmd(nc, in_maps, list(range(N_CORES)), trace=trace)
    y = np.concatenate([_unblock_y(res.results[k]["y"])
                        for k in range(N_CORES)], axis=1)
    return y[None], res


def kernel(x, W, b):
    y, _ = run(x, W, b)
    return y
